# revision 1
# baseline (speedup 1.0000x reference)
"""Trainium2 Bass kernel for multi-head self-attention (nn_Attention), v2.

Sharding over 8 NeuronCores: core = (g, b), g = head-pair (2 heads), b = batch.
Each core: fp16 QKV projection for its 2 heads, attention over n=2048
(scores S_T = K Q^T in fp16; exp split between the ACT engine and a DVE int16
bit-trick; P.V in fp16), and the per-head fp16 output projection. Outputs are
UNNORMALIZED per-head Y_h = (P_h V_h)^T W_oh^T plus per-row denominators; the
host divides and sums heads (division commutes with the output projection).

Key engine choices (measured on hw):
- exp: 12 of 16 kv tiles per unit on ACT (1 col/cycle @1.2GHz, (N+352)/1.2 ns);
  4 tiles on DVE via i16(s*c1+c2) bitcast fp16 == 2^(..) piecewise-linear exp
  (~3% sawtooth error whose mean cancels in softmax; bias constant absorbs the
  softmax shift B0). GpSimd cannot read PSUM, so it only handles memsets.
- exp is shifted by B0=2.5 to keep headroom (fp8e4 = IEEE e4m3, max 240,
  overflow -> inf; fp16 path is safe to s_raw ~ +-60).
- All PSUM drainage (exp, q/k/V/OT/Y copies) is balanced ACT/DVE; output DMAs
  are batched per half-span to keep the Sync engine off the critical path.
- PE warmup matmuls on DMA-independent garbage release the HAM clock gate
  (default state is 1.2GHz; ~3.4us of sustained PE busy unlocks 2.4GHz).
- fp8e4 DoubleRow P.V (2 kv tiles per matmul, K=256) is implemented behind
  KERNEL_DR_PAIRS but off by default: it measured slower (pair-granular
  exp->PV dependency stalls) and costs ~1% extra error.
"""

import os

import numpy as np
import ml_dtypes

B, N, DIM = 2, 2048, 256
HEADS, D = 8, 64
INNER = HEADS * D
NH = 2
NT = N // 128  # 16 kv tiles
PAIRS = NT // 2
SPAN = 1024
NSP = N // SPAN
SUB = SPAN // 128
VP = 80  # padded V width (D + ones + zero pad, 16B-aligned for DoubleRow)
SCALE = D ** -0.5

# softmax shift: P~ = exp(s*SCALE - B0). Keeps fp8e4 (IEEE e4m3, max 240,
# overflow -> inf) clear of overflow; cancels exactly in the host division.
B0 = 2.5
# exp bit-trick constants (fp16): i16(s*C1 + C2) bitcast f16 ~= exp(s*SCALE-B0)
C1_16 = float(SCALE * np.log2(np.e) * 1024.0)
C2_16 = float(15.0 * 1024.0 - 0.045 * 1024.0 - B0 * np.log2(np.e) * 1024.0)

_CACHE = {}


def _build_nc():
    import concourse.mybir as mybir
    import concourse.tile as tile
    from concourse import bacc

    f32 = mybir.dt.float32
    f16 = mybir.dt.float16
    bf16 = mybir.dt.bfloat16
    i16 = mybir.dt.int16
    u8 = mybir.dt.uint8
    fp8 = mybir.dt.float8e4
    DR = mybir.MatmulPerfMode.DoubleRow
    EXPF = mybir.ActivationFunctionType.Exp

    dve_tiles = set(
        int(x)
        for x in os.environ.get("KERNEL_DVE_TILES", "3,7,10,13").split(",")
        if x != ""
    )
    assert 0 not in dve_tiles and NT - 1 not in dve_tiles
    dr_pairs = sorted(
        int(x) for x in os.environ.get("KERNEL_DR_PAIRS", "").split(",") if x != ""
    )
    dr_tiles = set(t for p in dr_pairs for t in (2 * p, 2 * p + 1))
    assert not (dr_tiles & dve_tiles)
    assert (not dr_pairs) or (0 in dr_pairs and PAIRS - 1 in dr_pairs)
    MPO = VP if dr_pairs else D + 1
    n_warm = int(os.environ.get("KERNEL_WARMUP", "14"))

    nc = bacc.Bacc("TRN2", num_devices=8)
    xT16 = nc.dram_tensor("xT16", [128, 2, N], f16, kind="ExternalInput")
    wq16 = nc.dram_tensor("wq16", [128, 2, NH * 192], f16, kind="ExternalInput")
    wo16 = nc.dram_tensor("wo16", [D, NH, DIM], f16, kind="ExternalInput")
    YH = nc.dram_tensor("YH", [NH, N, DIM], f16, kind="ExternalOutput")
    DEN = nc.dram_tensor("DEN", [NH, NSP, 1, SPAN], f16, kind="ExternalOutput")

    with tile.TileContext(nc) as tc:
        with (
            tc.tile_pool(name="const", bufs=1) as const,
            tc.tile_pool(name="pex8", bufs=4) as pex8p,
            tc.tile_pool(name="pex16", bufs=4) as pex16p,
            tc.tile_pool(name="otp", bufs=3) as otp,
            tc.tile_pool(name="y16", bufs=2) as y16p,
            tc.tile_pool(name="ps", bufs=2, space="PSUM") as ps,
            tc.tile_pool(name="po", bufs=1, space="PSUM") as po,
            tc.tile_pool(name="py", bufs=2, space="PSUM") as py,
        ):
            # ---- input DMAs (wq first: needed by every projection) ----------
            wq_sb = const.tile([128, 2, NH * 192], f16)
            nc.sync.dma_start(wq_sb, wq16[:])
            xT_sb = const.tile([128, 2, N], f16)
            for hblk in range(2):
                nc.sync.dma_start(
                    xT_sb[:, :, hblk * 1024 : (hblk + 1) * 1024],
                    xT16[:, :, hblk * 1024 : (hblk + 1) * 1024],
                )
            wo_sb = const.tile([D, NH, DIM], f16)
            nc.sync.dma_start(wo_sb, wo16[:])

            # ---- PE clock warmup on DMA-independent garbage -----------------
            g16 = const.tile([128, 512], bf16)
            nc.gpsimd.memset(g16, 0.5)
            for _ in range(n_warm):
                pw = ps.tile([128, 512], f32, tag="S", name="pwarm")
                nc.tensor.matmul(pw, g16[:, 0:128], g16, start=True, stop=True)

            # ACT exp table warm (~2.7us) while DMAs run
            warm = y16p.tile([64, 4], f32, name="actwarm")
            nc.vector.memset(warm, 0.0)
            nc.scalar.activation(warm, warm, EXPF)

            # ---- V: fp16 (+ fp8 DoubleRow pairs), ones col for denominator --
            V16 = const.tile([128, NH, NT, D + 1], f16)
            nc.vector.memset(V16[:, :, :, D : D + 1], 1.0)
            if dr_pairs:
                V2 = const.tile([128, NH, PAIRS, 2, VP], fp8)
                nc.vector.memset(V2[:, :, :, :, D : D + 1], 1.0)
                nc.vector.memset(V2[:, :, :, :, D + 1 : VP], 0.0)

            qT = const.tile([D, NH, N], f16)
            kT = const.tile([D, NH, N], f16)
            nbias = const.tile([128, 1], f32)
            nc.vector.memset(nbias, -B0)

            # ---- projections (fp8 DoubleRow, K=256 one pass) ----------------
            copy_eng = [0]

            def psum_copy(dst, src):
                # keep the ACT queue free for the critical exp chain
                nc.vector.tensor_copy(dst, src)
                copy_eng[0] += 1

            def emit_qk(hh, dst, off, blk):
                pp = py.tile([D, 512], f32, tag="Y", name="pp")
                for c in range(2):
                    nc.tensor.matmul(
                        pp,
                        wq_sb[:, c, hh * 192 + off : hh * 192 + off + D],
                        xT_sb[:, c, blk * 512 : (blk + 1) * 512],
                        start=(c == 0),
                        stop=(c == 1),
                    )
                psum_copy(dst[:, hh, blk * 512 : (blk + 1) * 512], pp)

            # wq v-slices of both heads as one strided AP: [128, c, hh, 64]
            wqv = wq_sb.rearrange("p c (h m) -> p c h m", h=NH)[:, :, :, 2 * D : 3 * D]

            def emit_v(blk):
                # V for BOTH heads, tiles blk*4..blk*4+3, N=128 per matmul
                pvb = py.tile([128, 2, 2, D], f32, tag="Y", name="pvb")
                for ti2 in range(2):
                    for c in range(2):
                        nc.tensor.matmul(
                            pvb[:, ti2, :, :],
                            xT_sb[:, c, (blk * 4 + ti2 * 2) * 128 : (blk * 4 + ti2 * 2 + 1) * 128],
                            wqv[:, c, :, :],
                            start=(c == 0),
                            stop=(c == 1),
                        )
                    t0 = blk * 4 + ti2 * 2
                    if t0 in dr_tiles:
                        psum_copy(
                            V2.rearrange("p h pr i v -> p h (pr i) v")[:, :, t0, 0:D],
                            pvb[:, ti2, :, :],
                        )
                    else:
                        psum_copy(V16[:, :, t0, 0:D], pvb[:, ti2, :, :])
                pvb2 = py.tile([128, 2, 2, D], f32, tag="Y", name="pvb2")
                for ti2 in range(2):
                    for c in range(2):
                        nc.tensor.matmul(
                            pvb2[:, ti2, :, :],
                            xT_sb[:, c, (blk * 4 + ti2 * 2 + 1) * 128 : (blk * 4 + ti2 * 2 + 2) * 128],
                            wqv[:, c, :, :],
                            start=(c == 0),
                            stop=(c == 1),
                        )
                    t1 = blk * 4 + ti2 * 2 + 1
                    if t1 in dr_tiles:
                        psum_copy(
                            V2.rearrange("p h pr i v -> p h (pr i) v")[:, :, t1, 0:D],
                            pvb2[:, ti2, :, :],
                        )
                    else:
                        psum_copy(V16[:, :, t1, 0:D], pvb2[:, ti2, :, :])

            # upfront: enough to start unit (0,0)
            emit_qk(0, qT, 0, 0)
            emit_qk(0, qT, 0, 1)
            emit_qk(0, kT, D, 0)
            emit_v(0)

            background = [
                lambda: emit_qk(0, kT, D, 1),
                lambda: emit_v(1),
                lambda: emit_qk(0, kT, D, 2),
                lambda: emit_v(2),
                lambda: emit_qk(0, kT, D, 3),
                lambda: emit_v(3),
                lambda: emit_qk(0, qT, 0, 2),
                lambda: emit_qk(0, qT, 0, 3),
                None,
                None,
                None,
                None,
                None,
                None,
                None,
                None,
                # unit 1 ((1,0)) slots: project head 1 q/k
                lambda: emit_qk(1, qT, 0, 0),
                lambda: emit_qk(1, qT, 0, 1),
                lambda: emit_qk(1, kT, D, 0),
                lambda: emit_qk(1, kT, D, 1),
                lambda: emit_qk(1, kT, D, 2),
                lambda: emit_qk(1, kT, D, 3),
                lambda: emit_qk(1, qT, 0, 2),
                lambda: emit_qk(1, qT, 0, 3),
            ]

            # ---- attention units -------------------------------------------
            units = [(s, hh) for hh in range(NH) for s in range(NSP)]
            pending = None  # [ot_tile, hh, s, next_j]

            yeng = [0]

            def emit_y(pnd):
                ot_t, hh_p, s_p, j, ysp = pnd
                pyt = py.tile([128, DIM], f32, tag="Y", name="pyt")
                nc.tensor.matmul(
                    pyt,
                    ot_t[0:D, j * 128 : (j + 1) * 128],
                    wo_sb[:, hh_p, :],
                    start=True,
                    stop=True,
                )
                nc.vector.tensor_copy(ysp[:, j, :], pyt)
                yeng[0] += 1
                if j + 1 in (SUB // 2, SUB):
                    j0 = 0 if j + 1 == SUB // 2 else SUB // 2
                    nc.sync.dma_start(
                        YH[
                            hh_p,
                            s_p * SPAN + j0 * 128 : s_p * SPAN + (j + 1) * 128,
                            :,
                        ].rearrange("(j p) m -> p j m", p=128),
                        ysp[:, j0 : j + 1, :],
                    )
                pnd[3] = j + 1

            for s, hh in units:
                po_t = po.tile([MPO, SPAN], f32, tag="O", name="po_t")
                pS_t = {}
                pex8_cur = [None]

                def emit_st(t, s=s, hh=hh, pS_t=pS_t):
                    pS = ps.tile([128, SPAN], f32, tag="S", name="pS")
                    pS_t[t] = pS
                    for half in range(2):
                        nc.tensor.matmul(
                            pS[:, half * 512 : (half + 1) * 512],
                            kT[:, hh, t * 128 : (t + 1) * 128],
                            qT[
                                :,
                                hh,
                                s * SPAN + half * 512 : s * SPAN + (half + 1) * 512,
                            ],
                            start=True,
                            stop=True,
                        )

                emit_st(0)
                pex8_cur = [None]
                for t in range(NT):
                    if t + 1 < NT:
                        emit_st(t + 1)
                    p, i = t // 2, t % 2
                    mode = (
                        "dr" if t in dr_tiles else ("dve" if t in dve_tiles else "act")
                    )
                    if mode == "dve":
                        pex16 = pex16p.tile([128, SPAN], i16, name="pex16")
                        nc.vector.tensor_scalar(
                            pex16,
                            pS_t.pop(t),
                            C1_16,
                            C2_16,
                            mybir.AluOpType.mult,
                            mybir.AluOpType.add,
                        )
                        pex = pex16.bitcast(f16)
                    elif mode == "dr":
                        if i == 0:
                            pex8_cur[0] = pex8p.tile([128, 2, SPAN], fp8, name="pex8")
                        nc.scalar.activation(
                            pex8_cur[0][:, i, :],
                            pS_t.pop(t),
                            EXPF,
                            scale=SCALE,
                            bias=nbias,
                        )
                    else:
                        pexf = pex8p.tile([128, SPAN], f16, name="pex")
                        nc.scalar.activation(
                            pexf, pS_t.pop(t), EXPF, scale=SCALE, bias=nbias
                        )
                        pex = pexf
                    if background:
                        bg = background.pop(0)
                        if bg is not None:
                            bg()
                    if pending is not None and t >= 4 and pending[3] < SUB:
                        emit_y(pending)
                    if mode == "dr":
                        if i == 1:
                            for half in range(2):
                                nc.tensor.matmul(
                                    po_t[:, half * 512 : (half + 1) * 512],
                                    V2[:, hh, p, :, :],
                                    pex8_cur[0][:, :, half * 512 : (half + 1) * 512],
                                    start=(p == dr_pairs[0]),
                                    stop=(p == dr_pairs[-1]),
                                    perf_mode=DR,
                                )
                    else:
                        for half in range(2):
                            nc.tensor.matmul(
                                po_t[0 : D + 1, half * 512 : (half + 1) * 512],
                                V16[:, hh, t, :],
                                pex[:, half * 512 : (half + 1) * 512],
                                start=(not dr_pairs and t == 0),
                                stop=(not dr_pairs and t == NT - 1),
                            )
                if pending is not None:
                    while pending[3] < SUB:
                        emit_y(pending)
                # output head: O^T rows 0..63 + denominator row 64, fp16
                ot_t = otp.tile([D + 1, SPAN], f16, name="ot_t")
                nc.vector.tensor_copy(ot_t[:, 0 : SPAN // 2], po_t[0 : D + 1, 0 : SPAN // 2])
                nc.vector.tensor_copy(ot_t[:, SPAN // 2 :], po_t[0 : D + 1, SPAN // 2 :])
                nc.sync.dma_start(DEN[hh, s], ot_t[D : D + 1, :])
                ysp = y16p.tile([128, SUB, DIM], f16, name="ysp")
                pending = [ot_t, hh, s, 0, ysp]

            while pending[3] < SUB:
                emit_y(pending)
    nc.compile()
    return nc


def get_nc():
    key = (
        "nc_v2",
        os.environ.get("KERNEL_DVE_TILES", "3,7,10,13"),
        os.environ.get("KERNEL_DR_PAIRS", ""),
    )
    if key not in _CACHE:
        _CACHE[key] = _build_nc()
    return _CACHE[key]


def _to_f16(a):
    return np.ascontiguousarray(a.astype(np.float16))


def make_in_maps(x, w_qkv, w_out):
    x = np.asarray(x, dtype=np.float32)
    w_qkv = np.asarray(w_qkv, dtype=np.float32)
    w_out = np.asarray(w_out, dtype=np.float32)
    in_maps = []
    for core in range(8):
        g, b = core % 4, core // 4
        xT = x[b].T  # [256, 2048]
        xT16 = _to_f16(xT.reshape(2, 128, N).transpose(1, 0, 2))
        wslice = w_qkv[g * 384 : (g + 1) * 384]  # [384, 256]
        wq16 = _to_f16(wslice.T.reshape(2, 128, NH * 192).transpose(1, 0, 2))
        wo16 = np.ascontiguousarray(
            np.stack(
                [
                    w_out[:, g * 128 + h * D : g * 128 + (h + 1) * D].T
                    for h in range(NH)
                ],
                axis=1,
            ).astype(np.float16)
        )
        in_maps.append({"xT16": xT16, "wq16": wq16, "wo16": wo16})
    return in_maps


def gather(results, b_out):
    y = np.zeros((B, N, DIM), np.float32)
    for core in range(8):
        g, b = core % 4, core // 4
        yh = results[core]["YH"].astype(np.float32)  # [NH, N, DIM]
        den = results[core]["DEN"].astype(np.float32).reshape(NH, N)
        y[b] += yh[0] / den[0][:, None]
        y[b] += yh[1] / den[1][:, None]
    y += np.asarray(b_out, dtype=np.float32)[None, None, :]
    return y


def kernel(x, mask, w_qkv, w_out, b_out):
    if not os.environ.get("KERNEL_TRACE"):
        os.environ.setdefault("BASS_NEVER_TRACE", "1")
    from concourse.bass_utils import run_bass_kernel_spmd

    nc = get_nc()
    in_maps = make_in_maps(x, w_qkv, w_out)
    br = run_bass_kernel_spmd(nc, in_maps, core_ids=list(range(8)))
    _CACHE["last_br"] = br
    return gather(br.results, b_out)


def run_traced(x, mask, w_qkv, w_out, b_out, tmpdir, trace_cores=(0,)):
    from concourse.bass_utils import run_bass_kernel_spmd

    nc = get_nc()
    in_maps = make_in_maps(x, w_qkv, w_out)
    br = run_bass_kernel_spmd(
        nc,
        in_maps,
        core_ids=list(range(8)),
        trace=True,
        tmpdir=tmpdir,
        trace_cores=list(trace_cores),
    )
    return gather(br.results, b_out), br

